# revision 20
# baseline (speedup 1.0000x reference)
"""Trainium2 Bass kernel for the capsule-routing nn module (v2, fp16).

Math (per batch element b):
    u[i,j,d]   = sum_k W[i,j,d,k] * x[b,i,k]
    a_0        = 0 ; c_r = softmax_i(a_{r-1}) ; s_r = sum_i c_r * u
    v_r        = squash(s_r) ; a_r = a_{r-1} + sum_d v_r * u   (r = 1,2)
    out        = v_3

Mapping (B=256 sharded over 8 cores, 32 per core), all 2-byte fp16
on the wide paths (PE 1 cyc/row, DVE 2x mode), fp32 only in PSUM and
the tiny squash chain.  Softmax logits are max-subtracted before exp
so fp16 never overflows (and precision concentrates on the dominant
routing weights).

  stage-1 (PE):   G[(b,j), (i,k)] = sum_d v[b,j,d] * W[i,j,d,k]
                  block-diag-v lhsT (fp16) x resident W panels (fp16),
                  j in groups of 4; PSUM supertiles of 1024 cols.
  fold (DVE/Pool): P = G * xrep from PSUM (fp16 out), per-supertile
                  segmented k-reduce -> a[(b,j), i] fp16.
  softmax (DVE+PE+ACT): m = rowmax(a); t = a - m; transpose t;
                  exp on eviction -> e[(i), (j,b)] fp16; Z via ones-mm.
  s-step (PE):    s1: psum += xs[c,k].T @ ws[c,k]  (uniform round)
                  s2/s3: psA[(j',d),(j,b)] += W-panel.T @ (e*xs);
                  diag-extract via same-partition DVE copies.
  squash (fp32):  v = s|s| / (Z^2 + |s|^2) on [b,(j,d)]; v -> vbd fp16.
"""

import numpy as np
from contextlib import ExitStack

import concourse.bacc as bacc
import concourse.bass as bass
import concourse.tile as tile
from concourse import mybir
from concourse.bass_utils import run_bass_kernel_spmd
from concourse.masks import make_identity


F32 = mybir.dt.float32
F16 = mybir.dt.float16
ALU = mybir.AluOpType
ACTF = mybir.ActivationFunctionType
AX = mybir.AxisListType

# Problem shapes (hardcoded).
B_FULL, I, K = 256, 1152, 8
J, D = 10, 16
N_CORES = 8
B = B_FULL // N_CORES          # 32 per core
JD = J * D                     # 160
IK = I * K                     # 9216
NC_CHUNKS = I // 128           # 9  (i chunks of 128)
ST = 1024                      # stage-1 supertile cols
NST = IK // ST                 # 9 supertiles per j-group
# j groups of 4 (last group has j8,j9)
JG = [(0, 4), (4, 4), (8, 2)]

# engine assignment knobs (tuned from traces).
# fold route per supertile: 'dve' = DVE STT direct from PSUM (1x);
# 'act' = ACT evacuates PSUM->fp16, DVE multiplies in SBUF (2x).
# 'act' = ACT copy + DVE 2x mult; 'pool' = ACT copy + Pool mult (Pool idle here)
FOLD_ROUTE = ['act', 'act', 'pool', 'act', 'act', 'pool', 'act', 'act', 'act']
EXK_POOL_CHUNKS = (2, 5, 8)           # offload 3 chunks/round to idle Pool

_CACHE = {}


def _build_nc():
    """Build the Bass module once (same program for all cores)."""
    nc = bacc.Bacc("TRN2", target_bir_lowering=False, debug=False)

    # DRAM tensors (per-core shapes), all fp16
    wp_d = [nc.dram_tensor(f"wp{g}", [nj * D, IK], F16, kind="ExternalInput")
            for g, (_, nj) in enumerate(JG)]
    ws_d = nc.dram_tensor("ws", [128, NC_CHUNKS * K * JD], F16, kind="ExternalInput")
    xs_d = nc.dram_tensor("xs", [128, NC_CHUNKS * K * B], F16, kind="ExternalInput")
    xrep_d = nc.dram_tensor("xrep", [128, IK], F16, kind="ExternalInput")
    out_d = nc.dram_tensor("out", [B, J, D], F32, kind="ExternalOutput")

    with tile.TileContext(nc) as tc, ExitStack() as ctx:
        # ---------------- pools ----------------
        const_p = ctx.enter_context(tc.tile_pool(name="const", bufs=1))
        wsp = ctx.enter_context(tc.tile_pool(name="wsp", bufs=1))
        psum_g = ctx.enter_context(tc.tile_pool(name="psum_g", bufs=2, space="PSUM"))
        psum_t = ctx.enter_context(tc.tile_pool(name="psum_t", bufs=1, space="PSUM"))
        psum_s = ctx.enter_context(tc.tile_pool(name="psum_s", bufs=1, space="PSUM"))
        psbg_p = ctx.enter_context(tc.tile_pool(name="psbg", bufs=2))
        atile_p = ctx.enter_context(tc.tile_pool(name="atile", bufs=1))
        etile_p = ctx.enter_context(tc.tile_pool(name="etile", bufs=1))
        small = ctx.enter_context(tc.tile_pool(name="small", bufs=2))
        exk_p = ctx.enter_context(tc.tile_pool(name="exk", bufs=3))
        dl_p = ctx.enter_context(tc.tile_pool(name="dl", bufs=2))
        gc_p = ctx.enter_context(tc.tile_pool(name="gc", bufs=3))
        dram_p = ctx.enter_context(tc.tile_pool(name="dram", bufs=2, space="DRAM"))

        # ---------------- resident constants & loads ----------------
        identH = const_p.tile([128, 128], F16)
        make_identity(nc, identH)
        ident32 = const_p.tile([32, 32], F32)
        make_identity(nc, ident32)
        identF = const_p.tile([128, 128], F32)
        make_identity(nc, identF)
        ones_t = const_p.tile([128, 1], F16)
        nc.vector.memset(ones_t, 1.0)

        # s-round resident W / x (chunked DMAs so compute can start early)
        ws_t = wsp.tile([128, NC_CHUNKS * K * JD], F16)
        xs_t = wsp.tile([128, NC_CHUNKS * K * B], F16)
        wp_t = [wsp.tile([nj * D, IK], F16, tag=f"wp{g}", name=f"wp{g}")
                for g, (_, nj) in enumerate(JG)]
        xrep_t = wsp.tile([128, IK], F16)
        for c in range(NC_CHUNKS):
            nc.sync.dma_start(xs_t[:, c * K * B:(c + 1) * K * B],
                              xs_d[:, c * K * B:(c + 1) * K * B])
        for c in range(NC_CHUNKS):
            nc.sync.dma_start(ws_t[:, c * K * JD:(c + 1) * K * JD],
                              ws_d[:, c * K * JD:(c + 1) * K * JD])
        for st in range(NST):
            sl = slice(st * ST, (st + 1) * ST)
            nc.sync.dma_start(wp_t[0][:, sl], wp_d[0][:, sl])
            nc.sync.dma_start(xrep_t[:, sl], xrep_d[:, sl])
        for st in range(NST):
            sl = slice(st * ST, (st + 1) * ST)
            nc.sync.dma_start(wp_t[1][:, sl], wp_d[1][:, sl])
            nc.sync.dma_start(wp_t[2][:, sl], wp_d[2][:, sl])

        def ws_ck(c, k):   # [(i)128, (jd)160] fp16
            return ws_t[:, (c * K + k) * JD:(c * K + k + 1) * JD]

        def xs_ck(c, k):   # [(i)128, b] fp16
            return xs_t[:, (c * K + k) * B:(c * K + k + 1) * B]

        # logits a: [(b,j4)=128, i=1152] per j-group, fp16
        a1 = [atile_p.tile([128, I], F16, tag=f"a1_{g}", name=f"a1_{g}") for g in range(3)]
        a2 = [atile_p.tile([128, I], F16, tag=f"a2_{g}", name=f"a2_{g}") for g in range(3)]
        tl = [atile_p.tile([128, I], F16, tag=f"t_{g}", name=f"t_{g}") for g in range(3)]
        # e tiles: [(i)=128 per chunk, (j,b)=320] fp16
        e_t = [etile_p.tile([128, J * B], F16, tag=f"e_{c}", name=f"e_{c}")
               for c in range(NC_CHUNKS)]
        # vbd (stage-1 lhsT, block-diag), fp16; zeros persist between rounds
        vbd = [const_p.tile([nj * D, 128], F16, tag=f"vbd_{g}", name=f"vbd_{g}")
               for g, (_, nj) in enumerate(JG)]
        # pre-transpose staging [(jj,b)=128, (jl,d)<=64], block-diag in fp16
        vbd_s = [const_p.tile([128, nj * D], F16, tag=f"vbs_{g}", name=f"vbs_{g}")
                 for g, (_, nj) in enumerate(JG)]
        for g in range(3):
            nc.vector.memset(vbd[g], 0.0)
            nc.vector.memset(vbd_s[g], 0.0)
        # v / squash scratch
        vpan = small.tile([B, JD], F32, tag="vpan")
        z_jb = small.tile([B, J], F32, tag="z_jb")

        def squash_from(s_ap):
            """s_ap: [B=32, (j,d)=160] (psum or sbuf) -> vpan [B,160] fp32.

            v = s_raw * |s_raw| / (Z^2 + |s_raw|^2)  (squash, c=e/Z folded)
            """
            s2 = small.tile([B, JD], F32, tag="sq_s2")
            nc.scalar.activation(out=s2, in_=s_ap, func=ACTF.Square)
            n2 = small.tile([B, J], F32, tag="sq_n2")
            nc.vector.tensor_reduce(
                out=n2, in_=s2[:].rearrange("b (j d) -> b j d", j=J),
                axis=AX.X, op=ALU.add)
            nr = small.tile([B, J], F32, tag="sq_nr")
            nc.scalar.activation(out=nr, in_=n2, func=ACTF.Sqrt)
            z2 = small.tile([B, J], F32, tag="sq_z2")
            nc.vector.tensor_mul(z2, z_jb, z_jb)
            den = small.tile([B, J], F32, tag="sq_den")
            nc.vector.tensor_add(den, n2, z2)
            rden = small.tile([B, J], F32, tag="sq_rden")
            nc.vector.reciprocal(rden, den)
            sig = small.tile([B, J], F32, tag="sq_sig")
            nc.vector.tensor_mul(sig, nr, rden)
            sig_b = bass.AP(tensor=sig.tensor, offset=sig.offset,
                            ap=[sig.ap[0], [sig.ap[1][0], J], [0, D]])
            nc.vector.tensor_mul(
                vpan[:].rearrange("b (j d) -> b j d", j=J),
                s_ap.rearrange("b (j d) -> b j d", j=J), sig_b)

        def v_to_vbd():
            """vpan [B,160] fp32 -> block-diag staging (32-aligned DVE
            copies) -> one PE transpose -> vbd fp16."""
            for g, (j0, nj) in enumerate(JG):
                for jl in range(nj):
                    nc.vector.tensor_copy(
                        vbd_s[g][32 * jl:32 * (jl + 1), 16 * jl:16 * (jl + 1)],
                        vpan[:, 64 * g + 16 * jl:64 * g + 16 * (jl + 1)])
                vtp = psum_t.tile([64, 128], F16, tag="at", name=f"vtp_{g}")
                nc.tensor.transpose(vtp[:nj * D, :], vbd_s[g][:],
                                    identH[:, :])
                nc.vector.tensor_copy(vbd[g][:], vtp[:nj * D, :])

        def stage1_and_a(a_out, a_prev):
            """G = vbd.T @ wp ; P = G*xrep (cols (k,i)) ; TT-tree k-sum."""
            for g, (j0, nj) in enumerate(JG):
                psb = psbg_p.tile([128, IK], F16, tag="psbg", name="psb")
                for st in range(NST):
                    sl = slice(st * ST, (st + 1) * ST)
                    gp = psum_g.tile([128, ST], F32, tag="gp")
                    nc.tensor.matmul(gp[:, 0:512], vbd[g][:nj * D, :],
                                     wp_t[g][:, st * ST:st * ST + 512],
                                     start=True, stop=True)
                    nc.tensor.matmul(gp[:, 512:ST], vbd[g][:nj * D, :],
                                     wp_t[g][:, st * ST + 512:(st + 1) * ST],
                                     start=True, stop=True)
                    if FOLD_ROUTE[st] == 'dve':
                        nc.vector.scalar_tensor_tensor(
                            out=psb[:, sl], in0=gp[:], scalar=1.0,
                            in1=xrep_t[:, sl], op0=ALU.mult, op1=ALU.mult)
                    else:
                        gc = gc_p.tile([128, ST], F16, tag="gc")
                        nc.scalar.copy(gc[:], gp[:])
                        eng = nc.gpsimd if FOLD_ROUTE[st] == 'pool' else nc.vector
                        eng.tensor_tensor(
                            out=psb[:, sl], in0=gc[:], in1=xrep_t[:, sl],
                            op=ALU.mult)
                    # keep-warm: tiny PE op chained on this supertile's fold
                    tick = psum_t.tile([32, 32], F16, tag="at", name="tick")
                    nc.tensor.transpose(tick[:], psb[:, st * ST:st * ST + 32],
                                        identH[:, :32])
                # k-sum tree over contiguous 1152-col k-slices (fp16 2x)
                H = 4 * I   # 4608
                nc.vector.tensor_tensor(out=psb[:, 0:H], in0=psb[:, 0:H],
                                        in1=psb[:, H:2 * H], op=ALU.add)
                nc.vector.tensor_tensor(out=psb[:, 0:H // 2], in0=psb[:, 0:H // 2],
                                        in1=psb[:, H // 2:H], op=ALU.add)
                if a_prev is None:
                    nc.vector.tensor_tensor(out=a_out[g][:], in0=psb[:, 0:I],
                                            in1=psb[:, I:2 * I], op=ALU.add)
                else:
                    dl = dl_p.tile([128, I], F16, tag="dl", name="dl")
                    nc.vector.tensor_tensor(out=dl[:], in0=psb[:, 0:I],
                                            in1=psb[:, I:2 * I], op=ALU.add)
                    nc.vector.tensor_add(a_out[g][:], a_prev[g][:], dl[:])

        def exp_and_z(a_tiles):
            """m=rowmax(a); t=a-m; transpose; exp -> e_t; Z -> z_jb."""
            for g in range(3):
                m = small.tile([128, 1], F32, tag="amax")
                nc.vector.tensor_reduce(out=m, in_=a_tiles[g][:], axis=AX.X,
                                        op=ALU.max)
                nc.vector.tensor_scalar_sub(out=tl[g][:], in0=a_tiles[g][:],
                                            scalar1=m[:])
            for c in range(NC_CHUNKS):
                at2 = psum_t.tile([128, J * B], F16, tag="at", name=f"at2_{c}")
                for g, (j0, nj) in enumerate(JG):
                    nc.tensor.transpose(
                        at2[:, 128 * g:128 * g + nj * B],
                        tl[g][:, c * 128:(c + 1) * 128],
                        identH[:, :nj * B])
                nc.scalar.activation(out=e_t[c][:], in_=at2[:], func=ACTF.Exp)
            zp = psum_s.tile([1, J * B], F32, tag="zps", name="zp")
            for c in range(NC_CHUNKS):
                nc.tensor.matmul(zp[:], ones_t[:], e_t[c][:],
                                 start=(c == 0), stop=(c == NC_CHUNKS - 1))
            zs = small.tile([1, J * B], F32, tag="zs")
            nc.vector.tensor_copy(zs[:], zp[:])
            zdr = dram_p.tile([1, J * B], F32, tag="zdr")
            nc.sync.dma_start(zdr[:], zs[:])
            for j in range(J):
                nc.sync.dma_start(z_jb[:, j:j + 1], zdr[0:1, j * B:(j + 1) * B])

        def s_round_uniform():
            """s1_raw[b,(j,d)] = sum_{c,k} xs[c,k].T @ ws[c,k]; squash Z=I."""
            ps = psum_s.tile([B, JD], F32, tag="zps", name="ps")
            n = 0
            for c in range(NC_CHUNKS):
                for k in range(K):
                    nc.tensor.matmul(ps[:], xs_ck(c, k), ws_ck(c, k),
                                     start=(n == 0), stop=(n == NC_CHUNKS * K - 1))
                    n += 1
            squash_from(ps[:])

        def s_round_weighted(write_out):
            """s_raw via e-weighted matmuls with diag extract; squash with Z."""
            psA_t = psum_s.tile([128, 8 * B], F32, tag="ps_sA", name="psA_t")
            psB_t = psum_s.tile([32, 2 * B], F32, tag="ps_sB", name="psB_t")
            psA = psA_t[:]                  # [(j'8,d),(j0..7,b)]
            psB = psB_t[:]                  # [(j'2,d),(j8..9,b)]
            n = 0
            for c in range(NC_CHUNKS):
                # exk[(k,j,b)] = e[c][(j,b)] * xs[c][(k,b)]  (one fused op)
                exk = exk_p.tile([128, K * J * B], F16, tag="exk")
                e_src = bass.AP(tensor=e_t[c].tensor, offset=e_t[c].offset,
                                ap=[e_t[c].ap[0], [0, K], [B, J], [1, B]])
                x_base = xs_t[:, c * K * B:(c + 1) * K * B]
                x_src = bass.AP(tensor=x_base.tensor, offset=x_base.offset,
                                ap=[x_base.ap[0], [B, K], [0, J], [1, B]])
                eng = nc.gpsimd if c in EXK_POOL_CHUNKS else nc.vector
                eng.tensor_tensor(
                    out=exk[:].rearrange("p (k j b) -> p k j b", k=K, j=J),
                    in0=e_src, in1=x_src, op=ALU.mult)
                for k in range(K):
                    st_ = (n == 0)
                    sp = (n == NC_CHUNKS * K - 1)
                    wck = ws_ck(c, k)
                    o = k * J * B
                    nc.tensor.matmul(psA, wck[:, 0:128], exk[:, o:o + 8 * B],
                                     start=st_, stop=sp)
                    nc.tensor.matmul(psB, wck[:, 128:160],
                                     exk[:, o + 8 * B:o + J * B],
                                     start=st_, stop=sp)
                    n += 1
            # diag extract -> s-panels [(j,d), b] -> PE transpose -> sraw
            psA_s = small.tile([128, 8 * B], F32, tag="psA_s")
            nc.vector.tensor_copy(psA_s[:], psA)
            psB_s = small.tile([32, 2 * B], F32, tag="psB_s")
            nc.vector.tensor_copy(psB_s[:], psB)
            spanA = small.tile([128, B], F32, tag="spanA")
            spanB = small.tile([32, B], F32, tag="spanB")
            for jp in range(8):
                eng = nc.sync if jp % 2 == 0 else nc.scalar
                eng.dma_start(
                    spanA[16 * jp:16 * (jp + 1), :],
                    psA_s[16 * jp:16 * (jp + 1), jp * B:(jp + 1) * B])
            for jp in range(2):
                eng = nc.sync if jp % 2 == 0 else nc.scalar
                eng.dma_start(
                    spanB[16 * jp:16 * (jp + 1), :],
                    psB_s[16 * jp:16 * (jp + 1), jp * B:(jp + 1) * B])
            stA = psum_t.tile([B, 128], F32, tag="at", name="stA")
            nc.tensor.transpose(stA[:], spanA[:], identF[:, :])
            stB = psum_t.tile([B, 32], F32, tag="at", name="stB")
            nc.tensor.transpose(stB[:], spanB[:], ident32[:, :])
            sraw = small.tile([B, JD], F32, tag="sraw")
            nc.vector.tensor_copy(sraw[:, 0:128], stA[:])
            nc.vector.tensor_copy(sraw[:, 128:160], stB[:])
            squash_from(sraw[:])
            if write_out:
                nc.sync.dma_start(
                    out_d[:, :, :].rearrange("b j d -> b (j d)"), vpan[:])

        # ================= program =================
        nc.vector.memset(z_jb, float(I))   # Z = I for the uniform round
        s_round_uniform()          # -> vpan = v1
        v_to_vbd()
        stage1_and_a(a1, None)     # a1
        exp_and_z(a1)              # e = exp(a1 - max), Z
        s_round_weighted(False)    # -> vpan = v2
        v_to_vbd()
        stage1_and_a(a2, a1)       # a2 = a1 + delta
        exp_and_z(a2)
        s_round_weighted(True)     # -> v3 -> out

    nc.finalize()
    return nc


def _prep_inputs(x_full, w_full):
    """Host-side layout prep (numpy, layout only). Returns per-core in_maps."""
    W = w_full  # [I, J, D, K]
    # wp[g]: [(j_loc,d), (i,k)] fp16
    wp = []
    for (j0, nj) in JG:
        wpg = W[:, j0:j0 + nj, :, :].transpose(1, 2, 3, 0).reshape(nj * D, IK)
        wp.append(np.ascontiguousarray(wpg, dtype=np.float16))
    # ws: [(i)128, c, k, (j,d)] fp16
    ws = W.reshape(NC_CHUNKS, 128, J, D, K).transpose(1, 0, 4, 2, 3)
    ws = np.ascontiguousarray(ws.reshape(128, NC_CHUNKS * K * JD), dtype=np.float16)

    in_maps = []
    for c in range(N_CORES):
        xb = x_full[c * B:(c + 1) * B]           # [32, I, K]
        xs = xb.reshape(B, NC_CHUNKS, 128, K).transpose(2, 1, 3, 0)  # [i,c,k,b]
        xs = np.ascontiguousarray(xs.reshape(128, NC_CHUNKS * K * B),
                                  dtype=np.float16)
        xki = xb.transpose(0, 2, 1).reshape(B, IK)      # [b, (k,i)]
        xrep = np.tile(xki, (4, 1)).astype(np.float16)
        m = {"ws": ws, "xs": xs, "xrep": np.ascontiguousarray(xrep)}
        for g in range(3):
            m[f"wp{g}"] = wp[g]
        in_maps.append(m)
    return in_maps


def kernel(x, W):
    """x: [256, 1152, 8] f32, W: [1152, 10, 16, 8] f32 -> [256, 10, 16] f32."""
    x = np.asarray(x, dtype=np.float32)
    W = np.asarray(W, dtype=np.float32)
    if "nc" not in _CACHE:
        _CACHE["nc"] = _build_nc()
    nc = _CACHE["nc"]
    in_maps = _prep_inputs(x, W)
    res = run_bass_kernel_spmd(nc, in_maps, core_ids=list(range(N_CORES)))
    outs = [r["out"] for r in res.results]
    return np.concatenate(outs, axis=0)


# revision 21
# speedup vs baseline: 1.0241x; 1.0241x over previous
"""Trainium2 Bass kernel for the capsule-routing nn module (v2, fp16).

Math (per batch element b):
    u[i,j,d]   = sum_k W[i,j,d,k] * x[b,i,k]
    a_0        = 0 ; c_r = softmax_i(a_{r-1}) ; s_r = sum_i c_r * u
    v_r        = squash(s_r) ; a_r = a_{r-1} + sum_d v_r * u   (r = 1,2)
    out        = v_3

Mapping (B=256 sharded over 8 cores, 32 per core), all 2-byte fp16
on the wide paths (PE 1 cyc/row, DVE 2x mode), fp32 only in PSUM and
the tiny squash chain.  Softmax logits are max-subtracted before exp
so fp16 never overflows (and precision concentrates on the dominant
routing weights).

  stage-1 (PE):   G[(b,j), (i,k)] = sum_d v[b,j,d] * W[i,j,d,k]
                  block-diag-v lhsT (fp16) x resident W panels (fp16),
                  j in groups of 4; PSUM supertiles of 1024 cols.
  fold (DVE/Pool): P = G * xrep from PSUM (fp16 out), per-supertile
                  segmented k-reduce -> a[(b,j), i] fp16.
  softmax (DVE+PE+ACT): m = rowmax(a); t = a - m; transpose t;
                  exp on eviction -> e[(i), (j,b)] fp16; Z via ones-mm.
  s-step (PE):    s1: psum += xs[c,k].T @ ws[c,k]  (uniform round)
                  s2/s3: psA[(j',d),(j,b)] += W-panel.T @ (e*xs);
                  diag-extract via same-partition DVE copies.
  squash (fp32):  v = s|s| / (Z^2 + |s|^2) on [b,(j,d)]; v -> vbd fp16.
"""

import numpy as np
from contextlib import ExitStack

import concourse.bacc as bacc
import concourse.bass as bass
import concourse.tile as tile
from concourse import mybir
from concourse.bass_utils import run_bass_kernel_spmd
from concourse.masks import make_identity


F32 = mybir.dt.float32
F16 = mybir.dt.float16
ALU = mybir.AluOpType
ACTF = mybir.ActivationFunctionType
AX = mybir.AxisListType

# Problem shapes (hardcoded).
B_FULL, I, K = 256, 1152, 8
J, D = 10, 16
N_CORES = 8
B = B_FULL // N_CORES          # 32 per core
JD = J * D                     # 160
IK = I * K                     # 9216
NC_CHUNKS = I // 128           # 9  (i chunks of 128)
ST = 1024                      # stage-1 supertile cols
NST = IK // ST                 # 9 supertiles per j-group
# j groups of 4 (last group has j8,j9)
JG = [(0, 4), (4, 4), (8, 2)]

# engine assignment knobs (tuned from traces).
# fold route per supertile: 'dve' = DVE STT direct from PSUM (1x);
# 'act' = ACT evacuates PSUM->fp16, DVE multiplies in SBUF (2x).
FOLD_ROUTE = ['act'] * 9              # ACT evacuates, DVE multiplies at 2x
EXK_POOL_CHUNKS = (2, 5, 8)           # offload 3 chunks/round to idle Pool

_CACHE = {}


def _build_nc():
    """Build the Bass module once (same program for all cores)."""
    nc = bacc.Bacc("TRN2", target_bir_lowering=False, debug=False)

    # DRAM tensors (per-core shapes), all fp16
    wp_d = [nc.dram_tensor(f"wp{g}", [nj * D, IK], F16, kind="ExternalInput")
            for g, (_, nj) in enumerate(JG)]
    ws_d = nc.dram_tensor("ws", [128, NC_CHUNKS * K * JD], F16, kind="ExternalInput")
    xs_d = nc.dram_tensor("xs", [128, NC_CHUNKS * K * B], F16, kind="ExternalInput")
    xrep_d = nc.dram_tensor("xrep", [128, IK], F16, kind="ExternalInput")
    out_d = nc.dram_tensor("out", [B, J, D], F32, kind="ExternalOutput")

    with tile.TileContext(nc) as tc, ExitStack() as ctx:
        # ---------------- pools ----------------
        const_p = ctx.enter_context(tc.tile_pool(name="const", bufs=1))
        wsp = ctx.enter_context(tc.tile_pool(name="wsp", bufs=1))
        psum_g = ctx.enter_context(tc.tile_pool(name="psum_g", bufs=2, space="PSUM"))
        psum_t = ctx.enter_context(tc.tile_pool(name="psum_t", bufs=1, space="PSUM"))
        psum_s = ctx.enter_context(tc.tile_pool(name="psum_s", bufs=1, space="PSUM"))
        psbg_p = ctx.enter_context(tc.tile_pool(name="psbg", bufs=2))
        atile_p = ctx.enter_context(tc.tile_pool(name="atile", bufs=1))
        etile_p = ctx.enter_context(tc.tile_pool(name="etile", bufs=1))
        small = ctx.enter_context(tc.tile_pool(name="small", bufs=2))
        exk_p = ctx.enter_context(tc.tile_pool(name="exk", bufs=3))
        dl_p = ctx.enter_context(tc.tile_pool(name="dl", bufs=2))
        gc_p = ctx.enter_context(tc.tile_pool(name="gc", bufs=3))
        dram_p = ctx.enter_context(tc.tile_pool(name="dram", bufs=2, space="DRAM"))

        # ---------------- resident constants & loads ----------------
        identH = const_p.tile([128, 128], F16)
        make_identity(nc, identH)
        ident32 = const_p.tile([32, 32], F32)
        make_identity(nc, ident32)
        identF = const_p.tile([128, 128], F32)
        make_identity(nc, identF)
        ones_t = const_p.tile([128, 1], F16)
        nc.vector.memset(ones_t, 1.0)

        # s-round resident W / x (chunked DMAs so compute can start early)
        ws_t = wsp.tile([128, NC_CHUNKS * K * JD], F16)
        xs_t = wsp.tile([128, NC_CHUNKS * K * B], F16)
        wp_t = [wsp.tile([nj * D, IK], F16, tag=f"wp{g}", name=f"wp{g}")
                for g, (_, nj) in enumerate(JG)]
        xrep_t = wsp.tile([128, IK], F16)
        for c in range(NC_CHUNKS):
            eng = nc.sync if c % 2 == 0 else nc.scalar
            eng.dma_start(xs_t[:, c * K * B:(c + 1) * K * B],
                          xs_d[:, c * K * B:(c + 1) * K * B])
        for c in range(NC_CHUNKS):
            eng = nc.sync if c % 2 == 0 else nc.scalar
            eng.dma_start(ws_t[:, c * K * JD:(c + 1) * K * JD],
                          ws_d[:, c * K * JD:(c + 1) * K * JD])
        for st in range(NST):
            sl = slice(st * ST, (st + 1) * ST)
            nc.sync.dma_start(wp_t[0][:, sl], wp_d[0][:, sl])
            nc.scalar.dma_start(xrep_t[:, sl], xrep_d[:, sl])
        for st in range(NST):
            sl = slice(st * ST, (st + 1) * ST)
            nc.sync.dma_start(wp_t[1][:, sl], wp_d[1][:, sl])
            nc.scalar.dma_start(wp_t[2][:, sl], wp_d[2][:, sl])

        def ws_ck(c, k):   # [(i)128, (jd)160] fp16
            return ws_t[:, (c * K + k) * JD:(c * K + k + 1) * JD]

        def xs_ck(c, k):   # [(i)128, b] fp16
            return xs_t[:, (c * K + k) * B:(c * K + k + 1) * B]

        # logits a: [(b,j4)=128, i=1152] per j-group, fp16
        a1 = [atile_p.tile([128, I], F16, tag=f"a1_{g}", name=f"a1_{g}") for g in range(3)]
        a2 = [atile_p.tile([128, I], F16, tag=f"a2_{g}", name=f"a2_{g}") for g in range(3)]
        tl = [atile_p.tile([128, I], F16, tag=f"t_{g}", name=f"t_{g}") for g in range(3)]
        # e tiles: [(i)=128 per chunk, (j,b)=320] fp16
        e_t = [etile_p.tile([128, J * B], F16, tag=f"e_{c}", name=f"e_{c}")
               for c in range(NC_CHUNKS)]
        # vbd (stage-1 lhsT, block-diag), fp16; zeros persist between rounds
        vbd = [const_p.tile([nj * D, 128], F16, tag=f"vbd_{g}", name=f"vbd_{g}")
               for g, (_, nj) in enumerate(JG)]
        # pre-transpose staging [(jj,b)=128, (jl,d)<=64], block-diag in fp16
        vbd_s = [const_p.tile([128, nj * D], F16, tag=f"vbs_{g}", name=f"vbs_{g}")
                 for g, (_, nj) in enumerate(JG)]
        for g in range(3):
            nc.vector.memset(vbd[g], 0.0)
            nc.vector.memset(vbd_s[g], 0.0)
        # v / squash scratch
        vpan = small.tile([B, JD], F32, tag="vpan")
        z_jb = small.tile([B, J], F32, tag="z_jb")

        def squash_from(s_ap):
            """s_ap: [B=32, (j,d)=160] (psum or sbuf) -> vpan [B,160] fp32.

            v = s_raw * |s_raw| / (Z^2 + |s_raw|^2)  (squash, c=e/Z folded)
            """
            s2 = small.tile([B, JD], F32, tag="sq_s2")
            nc.scalar.activation(out=s2, in_=s_ap, func=ACTF.Square)
            n2 = small.tile([B, J], F32, tag="sq_n2")
            nc.vector.tensor_reduce(
                out=n2, in_=s2[:].rearrange("b (j d) -> b j d", j=J),
                axis=AX.X, op=ALU.add)
            nr = small.tile([B, J], F32, tag="sq_nr")
            nc.scalar.activation(out=nr, in_=n2, func=ACTF.Sqrt)
            z2 = small.tile([B, J], F32, tag="sq_z2")
            nc.vector.tensor_mul(z2, z_jb, z_jb)
            den = small.tile([B, J], F32, tag="sq_den")
            nc.vector.tensor_add(den, n2, z2)
            rden = small.tile([B, J], F32, tag="sq_rden")
            nc.vector.reciprocal(rden, den)
            sig = small.tile([B, J], F32, tag="sq_sig")
            nc.vector.tensor_mul(sig, nr, rden)
            sig_b = bass.AP(tensor=sig.tensor, offset=sig.offset,
                            ap=[sig.ap[0], [sig.ap[1][0], J], [0, D]])
            nc.vector.tensor_mul(
                vpan[:].rearrange("b (j d) -> b j d", j=J),
                s_ap.rearrange("b (j d) -> b j d", j=J), sig_b)

        def v_to_vbd():
            """vpan [B,160] fp32 -> block-diag staging (32-aligned DVE
            copies) -> one PE transpose -> vbd fp16."""
            for g, (j0, nj) in enumerate(JG):
                for jl in range(nj):
                    nc.vector.tensor_copy(
                        vbd_s[g][32 * jl:32 * (jl + 1), 16 * jl:16 * (jl + 1)],
                        vpan[:, 64 * g + 16 * jl:64 * g + 16 * (jl + 1)])
                vtp = psum_t.tile([64, 128], F16, tag="at", name=f"vtp_{g}")
                nc.tensor.transpose(vtp[:nj * D, :], vbd_s[g][:],
                                    identH[:, :])
                nc.vector.tensor_copy(vbd[g][:], vtp[:nj * D, :])

        def stage1_and_a(a_out, a_prev):
            """G = vbd.T @ wp ; P = G*xrep (cols (k,i)) ; TT-tree k-sum."""
            for g, (j0, nj) in enumerate(JG):
                psb = psbg_p.tile([128, IK], F16, tag="psbg", name="psb")
                for st in range(NST):
                    sl = slice(st * ST, (st + 1) * ST)
                    gp = psum_g.tile([128, ST], F32, tag="gp")
                    nc.tensor.matmul(gp[:, 0:512], vbd[g][:nj * D, :],
                                     wp_t[g][:, st * ST:st * ST + 512],
                                     start=True, stop=True)
                    nc.tensor.matmul(gp[:, 512:ST], vbd[g][:nj * D, :],
                                     wp_t[g][:, st * ST + 512:(st + 1) * ST],
                                     start=True, stop=True)
                    if FOLD_ROUTE[st] == 'dve':
                        nc.vector.scalar_tensor_tensor(
                            out=psb[:, sl], in0=gp[:], scalar=1.0,
                            in1=xrep_t[:, sl], op0=ALU.mult, op1=ALU.mult)
                    else:
                        gc = gc_p.tile([128, ST], F16, tag="gc")
                        nc.scalar.copy(gc[:], gp[:])
                        eng = nc.gpsimd if FOLD_ROUTE[st] == 'pool' else nc.vector
                        eng.tensor_tensor(
                            out=psb[:, sl], in0=gc[:], in1=xrep_t[:, sl],
                            op=ALU.mult)
                # k-sum tree over contiguous 1152-col k-slices (fp16 2x)
                H = 4 * I   # 4608
                nc.vector.tensor_tensor(out=psb[:, 0:H], in0=psb[:, 0:H],
                                        in1=psb[:, H:2 * H], op=ALU.add)
                nc.vector.tensor_tensor(out=psb[:, 0:H // 2], in0=psb[:, 0:H // 2],
                                        in1=psb[:, H // 2:H], op=ALU.add)
                if a_prev is None:
                    nc.vector.tensor_tensor(out=a_out[g][:], in0=psb[:, 0:I],
                                            in1=psb[:, I:2 * I], op=ALU.add)
                else:
                    dl = dl_p.tile([128, I], F16, tag="dl", name="dl")
                    nc.vector.tensor_tensor(out=dl[:], in0=psb[:, 0:I],
                                            in1=psb[:, I:2 * I], op=ALU.add)
                    nc.vector.tensor_add(a_out[g][:], a_prev[g][:], dl[:])

        def exp_and_z(a_tiles):
            """m=rowmax(a); t=a-m; transpose; exp -> e_t; Z -> z_jb."""
            for g in range(3):
                m = small.tile([128, 1], F32, tag="amax")
                nc.vector.tensor_reduce(out=m, in_=a_tiles[g][:], axis=AX.X,
                                        op=ALU.max)
                nc.vector.tensor_scalar_sub(out=tl[g][:], in0=a_tiles[g][:],
                                            scalar1=m[:])
            for c in range(NC_CHUNKS):
                at2 = psum_t.tile([128, J * B], F16, tag="at", name=f"at2_{c}")
                for g, (j0, nj) in enumerate(JG):
                    nc.tensor.transpose(
                        at2[:, 128 * g:128 * g + nj * B],
                        tl[g][:, c * 128:(c + 1) * 128],
                        identH[:, :nj * B])
                nc.scalar.activation(out=e_t[c][:], in_=at2[:], func=ACTF.Exp)
            zp = psum_s.tile([1, J * B], F32, tag="zps", name="zp")
            for c in range(NC_CHUNKS):
                nc.tensor.matmul(zp[:], ones_t[:], e_t[c][:],
                                 start=(c == 0), stop=(c == NC_CHUNKS - 1))
            zs = small.tile([1, J * B], F32, tag="zs")
            nc.vector.tensor_copy(zs[:], zp[:])
            zdr = dram_p.tile([1, J * B], F32, tag="zdr")
            nc.sync.dma_start(zdr[:], zs[:])
            for j in range(J):
                nc.sync.dma_start(z_jb[:, j:j + 1], zdr[0:1, j * B:(j + 1) * B])

        def s_round_uniform():
            """s1_raw[b,(j,d)] = sum_{c,k} xs[c,k].T @ ws[c,k]; squash Z=I."""
            ps = psum_s.tile([B, JD], F32, tag="zps", name="ps")
            n = 0
            for c in range(NC_CHUNKS):
                for k in range(K):
                    nc.tensor.matmul(ps[:], xs_ck(c, k), ws_ck(c, k),
                                     start=(n == 0), stop=(n == NC_CHUNKS * K - 1))
                    n += 1
            squash_from(ps[:])

        def s_round_weighted(write_out):
            """s_raw via e-weighted matmuls with diag extract; squash with Z."""
            psA_t = psum_s.tile([128, 8 * B], F32, tag="ps_sA", name="psA_t")
            psB_t = psum_s.tile([32, 2 * B], F32, tag="ps_sB", name="psB_t")
            psA = psA_t[:]                  # [(j'8,d),(j0..7,b)]
            psB = psB_t[:]                  # [(j'2,d),(j8..9,b)]
            n = 0
            for c in range(NC_CHUNKS):
                # exk[(k,j,b)] = e[c][(j,b)] * xs[c][(k,b)]  (one fused op)
                exk = exk_p.tile([128, K * J * B], F16, tag="exk")
                e_src = bass.AP(tensor=e_t[c].tensor, offset=e_t[c].offset,
                                ap=[e_t[c].ap[0], [0, K], [B, J], [1, B]])
                x_base = xs_t[:, c * K * B:(c + 1) * K * B]
                x_src = bass.AP(tensor=x_base.tensor, offset=x_base.offset,
                                ap=[x_base.ap[0], [B, K], [0, J], [1, B]])
                eng = nc.gpsimd if c in EXK_POOL_CHUNKS else nc.vector
                eng.tensor_tensor(
                    out=exk[:].rearrange("p (k j b) -> p k j b", k=K, j=J),
                    in0=e_src, in1=x_src, op=ALU.mult)
                for k in range(K):
                    st_ = (n == 0)
                    sp = (n == NC_CHUNKS * K - 1)
                    wck = ws_ck(c, k)
                    o = k * J * B
                    nc.tensor.matmul(psA, wck[:, 0:128], exk[:, o:o + 8 * B],
                                     start=st_, stop=sp)
                    nc.tensor.matmul(psB, wck[:, 128:160],
                                     exk[:, o + 8 * B:o + J * B],
                                     start=st_, stop=sp)
                    n += 1
            # diag extract -> s-panels [(j,d), b] -> PE transpose -> sraw
            psA_s = small.tile([128, 8 * B], F32, tag="psA_s")
            nc.vector.tensor_copy(psA_s[:], psA)
            psB_s = small.tile([32, 2 * B], F32, tag="psB_s")
            nc.vector.tensor_copy(psB_s[:], psB)
            spanA = small.tile([128, B], F32, tag="spanA")
            spanB = small.tile([32, B], F32, tag="spanB")
            for jp in range(8):
                eng = nc.sync if jp % 2 == 0 else nc.scalar
                eng.dma_start(
                    spanA[16 * jp:16 * (jp + 1), :],
                    psA_s[16 * jp:16 * (jp + 1), jp * B:(jp + 1) * B])
            for jp in range(2):
                eng = nc.sync if jp % 2 == 0 else nc.scalar
                eng.dma_start(
                    spanB[16 * jp:16 * (jp + 1), :],
                    psB_s[16 * jp:16 * (jp + 1), jp * B:(jp + 1) * B])
            stA = psum_t.tile([B, 128], F32, tag="at", name="stA")
            nc.tensor.transpose(stA[:], spanA[:], identF[:, :])
            stB = psum_t.tile([B, 32], F32, tag="at", name="stB")
            nc.tensor.transpose(stB[:], spanB[:], ident32[:, :])
            sraw = small.tile([B, JD], F32, tag="sraw")
            nc.vector.tensor_copy(sraw[:, 0:128], stA[:])
            nc.vector.tensor_copy(sraw[:, 128:160], stB[:])
            squash_from(sraw[:])
            if write_out:
                nc.sync.dma_start(
                    out_d[:, :, :].rearrange("b j d -> b (j d)"), vpan[:])

        # ================= program =================
        nc.vector.memset(z_jb, float(I))   # Z = I for the uniform round
        s_round_uniform()          # -> vpan = v1
        v_to_vbd()
        stage1_and_a(a1, None)     # a1
        exp_and_z(a1)              # e = exp(a1 - max), Z
        s_round_weighted(False)    # -> vpan = v2
        v_to_vbd()
        stage1_and_a(a2, a1)       # a2 = a1 + delta
        exp_and_z(a2)
        s_round_weighted(True)     # -> v3 -> out

    nc.finalize()
    return nc


def _prep_inputs(x_full, w_full):
    """Host-side layout prep (numpy, layout only). Returns per-core in_maps."""
    W = w_full  # [I, J, D, K]
    # wp[g]: [(j_loc,d), (i,k)] fp16
    wp = []
    for (j0, nj) in JG:
        wpg = W[:, j0:j0 + nj, :, :].transpose(1, 2, 3, 0).reshape(nj * D, IK)
        wp.append(np.ascontiguousarray(wpg, dtype=np.float16))
    # ws: [(i)128, c, k, (j,d)] fp16
    ws = W.reshape(NC_CHUNKS, 128, J, D, K).transpose(1, 0, 4, 2, 3)
    ws = np.ascontiguousarray(ws.reshape(128, NC_CHUNKS * K * JD), dtype=np.float16)

    in_maps = []
    for c in range(N_CORES):
        xb = x_full[c * B:(c + 1) * B]           # [32, I, K]
        xs = xb.reshape(B, NC_CHUNKS, 128, K).transpose(2, 1, 3, 0)  # [i,c,k,b]
        xs = np.ascontiguousarray(xs.reshape(128, NC_CHUNKS * K * B),
                                  dtype=np.float16)
        xki = xb.transpose(0, 2, 1).reshape(B, IK)      # [b, (k,i)]
        xrep = np.tile(xki, (4, 1)).astype(np.float16)
        m = {"ws": ws, "xs": xs, "xrep": np.ascontiguousarray(xrep)}
        for g in range(3):
            m[f"wp{g}"] = wp[g]
        in_maps.append(m)
    return in_maps


def kernel(x, W):
    """x: [256, 1152, 8] f32, W: [1152, 10, 16, 8] f32 -> [256, 10, 16] f32."""
    x = np.asarray(x, dtype=np.float32)
    W = np.asarray(W, dtype=np.float32)
    if "nc" not in _CACHE:
        _CACHE["nc"] = _build_nc()
    nc = _CACHE["nc"]
    in_maps = _prep_inputs(x, W)
    res = run_bass_kernel_spmd(nc, in_maps, core_ids=list(range(N_CORES)))
    outs = [r["out"] for r in res.results]
    return np.concatenate(outs, axis=0)


# revision 22
# speedup vs baseline: 1.0273x; 1.0031x over previous
"""Trainium2 Bass kernel for the capsule-routing nn module (v2, fp16).

Math (per batch element b):
    u[i,j,d]   = sum_k W[i,j,d,k] * x[b,i,k]
    a_0        = 0 ; c_r = softmax_i(a_{r-1}) ; s_r = sum_i c_r * u
    v_r        = squash(s_r) ; a_r = a_{r-1} + sum_d v_r * u   (r = 1,2)
    out        = v_3

Mapping (B=256 sharded over 8 cores, 32 per core), all 2-byte fp16
on the wide paths (PE 1 cyc/row, DVE 2x mode), fp32 only in PSUM and
the tiny squash chain.  Softmax logits are max-subtracted before exp
so fp16 never overflows (and precision concentrates on the dominant
routing weights).

  stage-1 (PE):   G[(b,j), (i,k)] = sum_d v[b,j,d] * W[i,j,d,k]
                  block-diag-v lhsT (fp16) x resident W panels (fp16),
                  j in groups of 4; PSUM supertiles of 1024 cols.
  fold (DVE/Pool): P = G * xrep from PSUM (fp16 out), per-supertile
                  segmented k-reduce -> a[(b,j), i] fp16.
  softmax (DVE+PE+ACT): m = rowmax(a); t = a - m; transpose t;
                  exp on eviction -> e[(i), (j,b)] fp16; Z via ones-mm.
  s-step (PE):    s1: psum += xs[c,k].T @ ws[c,k]  (uniform round)
                  s2/s3: psA[(j',d),(j,b)] += W-panel.T @ (e*xs);
                  diag-extract via same-partition DVE copies.
  squash (fp32):  v = s|s| / (Z^2 + |s|^2) on [b,(j,d)]; v -> vbd fp16.
"""

import numpy as np
from contextlib import ExitStack

import concourse.bacc as bacc
import concourse.bass as bass
import concourse.tile as tile
from concourse import mybir
from concourse.bass_utils import run_bass_kernel_spmd
from concourse.masks import make_identity


F32 = mybir.dt.float32
F16 = mybir.dt.float16
ALU = mybir.AluOpType
ACTF = mybir.ActivationFunctionType
AX = mybir.AxisListType

# Problem shapes (hardcoded).
B_FULL, I, K = 256, 1152, 8
J, D = 10, 16
N_CORES = 8
B = B_FULL // N_CORES          # 32 per core
JD = J * D                     # 160
IK = I * K                     # 9216
NC_CHUNKS = I // 128           # 9  (i chunks of 128)
ST = 1024                      # stage-1 supertile cols
NST = IK // ST                 # 9 supertiles per j-group
# j groups of 4 (last group has j8,j9)
JG = [(0, 4), (4, 4), (8, 2)]

# engine assignment knobs (tuned from traces).
# fold route per supertile: 'dve' = DVE STT direct from PSUM (1x);
# 'act' = ACT evacuates PSUM->fp16, DVE multiplies in SBUF (2x).
FOLD_ROUTE = ['act'] * 9              # ACT evacuates, DVE multiplies at 2x
EXK_POOL_CHUNKS = (2, 5, 8)           # offload 3 chunks/round to idle Pool

_CACHE = {}


def _build_nc():
    """Build the Bass module once (same program for all cores)."""
    nc = bacc.Bacc("TRN2", target_bir_lowering=False, debug=False)

    # DRAM tensors (per-core shapes), all fp16
    wp_d = [nc.dram_tensor(f"wp{g}", [nj * D, IK], F16, kind="ExternalInput")
            for g, (_, nj) in enumerate(JG)]
    ws_d = nc.dram_tensor("ws", [128, NC_CHUNKS * K * JD], F16, kind="ExternalInput")
    xs_d = nc.dram_tensor("xs", [128, NC_CHUNKS * K * B], F16, kind="ExternalInput")
    xrep_d = nc.dram_tensor("xrep", [128, IK], F16, kind="ExternalInput")
    out_d = nc.dram_tensor("out", [B, J, D], F32, kind="ExternalOutput")

    with tile.TileContext(nc) as tc, ExitStack() as ctx:
        # ---------------- pools ----------------
        const_p = ctx.enter_context(tc.tile_pool(name="const", bufs=1))
        wsp = ctx.enter_context(tc.tile_pool(name="wsp", bufs=1))
        psum_g = ctx.enter_context(tc.tile_pool(name="psum_g", bufs=2, space="PSUM"))
        psum_t = ctx.enter_context(tc.tile_pool(name="psum_t", bufs=1, space="PSUM"))
        psum_s = ctx.enter_context(tc.tile_pool(name="psum_s", bufs=1, space="PSUM"))
        psbg_p = ctx.enter_context(tc.tile_pool(name="psbg", bufs=2))
        atile_p = ctx.enter_context(tc.tile_pool(name="atile", bufs=1))
        etile_p = ctx.enter_context(tc.tile_pool(name="etile", bufs=1))
        small = ctx.enter_context(tc.tile_pool(name="small", bufs=2))
        exk_p = ctx.enter_context(tc.tile_pool(name="exk", bufs=3))
        dl_p = ctx.enter_context(tc.tile_pool(name="dl", bufs=2))
        gc_p = ctx.enter_context(tc.tile_pool(name="gc", bufs=3))
        dram_p = ctx.enter_context(tc.tile_pool(name="dram", bufs=2, space="DRAM"))

        # ---------------- resident constants & loads ----------------
        identH = const_p.tile([128, 128], F16)
        make_identity(nc, identH)
        ident32 = const_p.tile([32, 32], F32)
        make_identity(nc, ident32)
        identF = const_p.tile([128, 128], F32)
        make_identity(nc, identF)
        ones_t = const_p.tile([128, 1], F16)
        nc.vector.memset(ones_t, 1.0)

        # s-round resident W / x (chunked DMAs so compute can start early)
        ws_t = wsp.tile([128, NC_CHUNKS * K * JD], F16)
        xs_t = wsp.tile([128, NC_CHUNKS * K * B], F16)
        wp_t = [wsp.tile([nj * D, IK], F16, tag=f"wp{g}", name=f"wp{g}")
                for g, (_, nj) in enumerate(JG)]
        xrep_t = wsp.tile([128, IK], F16)
        for c in range(NC_CHUNKS):
            eng = nc.sync if c % 2 == 0 else nc.scalar
            eng.dma_start(xs_t[:, c * K * B:(c + 1) * K * B],
                          xs_d[:, c * K * B:(c + 1) * K * B])
        for c in range(NC_CHUNKS):
            eng = nc.sync if c % 2 == 0 else nc.scalar
            eng.dma_start(ws_t[:, c * K * JD:(c + 1) * K * JD],
                          ws_d[:, c * K * JD:(c + 1) * K * JD])
        for st in range(NST):
            sl = slice(st * ST, (st + 1) * ST)
            nc.sync.dma_start(wp_t[0][:, sl], wp_d[0][:, sl])
            nc.scalar.dma_start(xrep_t[:, sl], xrep_d[:, sl])
        for st in range(NST):
            sl = slice(st * ST, (st + 1) * ST)
            nc.sync.dma_start(wp_t[1][:, sl], wp_d[1][:, sl])
            nc.scalar.dma_start(wp_t[2][:, sl], wp_d[2][:, sl])

        def ws_ck(c, k):   # [(i)128, (jd)160] fp16
            return ws_t[:, (c * K + k) * JD:(c * K + k + 1) * JD]

        def xs_ck(c, k):   # [(i)128, b] fp16
            return xs_t[:, (c * K + k) * B:(c * K + k + 1) * B]

        # logits a: [(b,j4)=128, i=1152] per j-group, fp16
        a1 = [atile_p.tile([128, I], F16, tag=f"a1_{g}", name=f"a1_{g}") for g in range(3)]
        a2 = [atile_p.tile([128, I], F16, tag=f"a2_{g}", name=f"a2_{g}") for g in range(3)]
        tl = [atile_p.tile([128, I], F16, tag=f"t_{g}", name=f"t_{g}") for g in range(3)]
        # e tiles: [(i)=128 per chunk, (j,b)=320] fp16
        e_t = [etile_p.tile([128, J * B], F16, tag=f"e_{c}", name=f"e_{c}")
               for c in range(NC_CHUNKS)]
        # vbd (stage-1 lhsT, block-diag), fp16; zeros persist between rounds
        vbd = [const_p.tile([nj * D, 128], F16, tag=f"vbd_{g}", name=f"vbd_{g}")
               for g, (_, nj) in enumerate(JG)]
        # pre-transpose staging [(jj,b)=128, (jl,d)<=64], block-diag in fp16
        vbd_s = [const_p.tile([128, nj * D], F16, tag=f"vbs_{g}", name=f"vbs_{g}")
                 for g, (_, nj) in enumerate(JG)]
        for g in range(3):
            nc.vector.memset(vbd[g], 0.0)
            nc.vector.memset(vbd_s[g], 0.0)
        # v / squash scratch
        vpan = small.tile([B, JD], F32, tag="vpan")
        z_jb = small.tile([B, J], F32, tag="z_jb")

        def squash_from(s_ap):
            """s_ap: [B=32, (j,d)=160] (psum or sbuf) -> vpan [B,160] fp32.

            v = s_raw * |s_raw| / (Z^2 + |s_raw|^2)  (squash, c=e/Z folded)
            """
            s2 = small.tile([B, JD], F32, tag="sq_s2")
            nc.scalar.activation(out=s2, in_=s_ap, func=ACTF.Square)
            n2 = small.tile([B, J], F32, tag="sq_n2")
            nc.vector.tensor_reduce(
                out=n2, in_=s2[:].rearrange("b (j d) -> b j d", j=J),
                axis=AX.X, op=ALU.add)
            nr = small.tile([B, J], F32, tag="sq_nr")
            nc.scalar.activation(out=nr, in_=n2, func=ACTF.Sqrt)
            z2 = small.tile([B, J], F32, tag="sq_z2")
            nc.vector.tensor_mul(z2, z_jb, z_jb)
            den = small.tile([B, J], F32, tag="sq_den")
            nc.vector.tensor_add(den, n2, z2)
            rden = small.tile([B, J], F32, tag="sq_rden")
            nc.vector.reciprocal(rden, den)
            sig = small.tile([B, J], F32, tag="sq_sig")
            nc.vector.tensor_mul(sig, nr, rden)
            sig_b = bass.AP(tensor=sig.tensor, offset=sig.offset,
                            ap=[sig.ap[0], [sig.ap[1][0], J], [0, D]])
            nc.vector.tensor_mul(
                vpan[:].rearrange("b (j d) -> b j d", j=J),
                s_ap.rearrange("b (j d) -> b j d", j=J), sig_b)

        def v_to_vbd():
            """vpan [B,160] fp32 -> block-diag staging (32-aligned DVE
            copies) -> one PE transpose -> vbd fp16."""
            for g, (j0, nj) in enumerate(JG):
                for jl in range(nj):
                    nc.vector.tensor_copy(
                        vbd_s[g][32 * jl:32 * (jl + 1), 16 * jl:16 * (jl + 1)],
                        vpan[:, 64 * g + 16 * jl:64 * g + 16 * (jl + 1)])
                vtp = psum_t.tile([64, 128], F16, tag="at", name=f"vtp_{g}")
                nc.tensor.transpose(vtp[:nj * D, :], vbd_s[g][:],
                                    identH[:, :])
                nc.vector.tensor_copy(vbd[g][:], vtp[:nj * D, :])

        def stage1_and_a(a_out, a_prev):
            """G = vbd.T @ wp ; P = G*xrep (cols (k,i)) ; TT-tree k-sum."""
            for g, (j0, nj) in enumerate(JG):
                psb = psbg_p.tile([128, IK], F16, tag="psbg", name="psb")
                for st in range(NST):
                    sl = slice(st * ST, (st + 1) * ST)
                    gp = psum_g.tile([128, ST], F32, tag="gp")
                    nc.tensor.matmul(gp[:, 0:512], vbd[g][:nj * D, :],
                                     wp_t[g][:, st * ST:st * ST + 512],
                                     start=True, stop=True)
                    nc.tensor.matmul(gp[:, 512:ST], vbd[g][:nj * D, :],
                                     wp_t[g][:, st * ST + 512:(st + 1) * ST],
                                     start=True, stop=True)
                    if FOLD_ROUTE[st] == 'dve':
                        nc.vector.scalar_tensor_tensor(
                            out=psb[:, sl], in0=gp[:], scalar=1.0,
                            in1=xrep_t[:, sl], op0=ALU.mult, op1=ALU.mult)
                    else:
                        gc = gc_p.tile([128, ST], F16, tag="gc")
                        nc.scalar.copy(gc[:], gp[:])
                        eng = nc.gpsimd if FOLD_ROUTE[st] == 'pool' else nc.vector
                        eng.tensor_tensor(
                            out=psb[:, sl], in0=gc[:], in1=xrep_t[:, sl],
                            op=ALU.mult)
                    # keep-warm: tiny PE op chained on this supertile's fold
                    tick = psum_t.tile([32, 32], F16, tag="at", name="tick")
                    nc.tensor.transpose(tick[:], psb[:, st * ST:st * ST + 32],
                                        identH[:, :32])
                # k-sum tree over contiguous 1152-col k-slices (fp16 2x)
                H = 4 * I   # 4608
                nc.vector.tensor_tensor(out=psb[:, 0:H], in0=psb[:, 0:H],
                                        in1=psb[:, H:2 * H], op=ALU.add)
                nc.vector.tensor_tensor(out=psb[:, 0:H // 2], in0=psb[:, 0:H // 2],
                                        in1=psb[:, H // 2:H], op=ALU.add)
                if a_prev is None:
                    nc.vector.tensor_tensor(out=a_out[g][:], in0=psb[:, 0:I],
                                            in1=psb[:, I:2 * I], op=ALU.add)
                else:
                    dl = dl_p.tile([128, I], F16, tag="dl", name="dl")
                    nc.vector.tensor_tensor(out=dl[:], in0=psb[:, 0:I],
                                            in1=psb[:, I:2 * I], op=ALU.add)
                    nc.vector.tensor_add(a_out[g][:], a_prev[g][:], dl[:])

        def exp_and_z(a_tiles):
            """m=rowmax(a); t=a-m; transpose; exp -> e_t; Z -> z_jb."""
            for g in range(3):
                m = small.tile([128, 1], F32, tag="amax")
                nc.vector.tensor_reduce(out=m, in_=a_tiles[g][:], axis=AX.X,
                                        op=ALU.max)
                nc.vector.tensor_scalar_sub(out=tl[g][:], in0=a_tiles[g][:],
                                            scalar1=m[:])
            for c in range(NC_CHUNKS):
                at2 = psum_t.tile([128, J * B], F16, tag="at", name=f"at2_{c}")
                for g, (j0, nj) in enumerate(JG):
                    nc.tensor.transpose(
                        at2[:, 128 * g:128 * g + nj * B],
                        tl[g][:, c * 128:(c + 1) * 128],
                        identH[:, :nj * B])
                nc.scalar.activation(out=e_t[c][:], in_=at2[:], func=ACTF.Exp)
            zp = psum_s.tile([1, J * B], F32, tag="zps", name="zp")
            for c in range(NC_CHUNKS):
                nc.tensor.matmul(zp[:], ones_t[:], e_t[c][:],
                                 start=(c == 0), stop=(c == NC_CHUNKS - 1))
            zs = small.tile([1, J * B], F32, tag="zs")
            nc.vector.tensor_copy(zs[:], zp[:])
            zdr = dram_p.tile([1, J * B], F32, tag="zdr")
            nc.sync.dma_start(zdr[:], zs[:])
            for j in range(J):
                nc.sync.dma_start(z_jb[:, j:j + 1], zdr[0:1, j * B:(j + 1) * B])

        def s_round_uniform():
            """s1_raw[b,(j,d)] = sum_{c,k} xs[c,k].T @ ws[c,k]; squash Z=I."""
            ps = psum_s.tile([B, JD], F32, tag="zps", name="ps")
            n = 0
            for c in range(NC_CHUNKS):
                for k in range(K):
                    nc.tensor.matmul(ps[:], xs_ck(c, k), ws_ck(c, k),
                                     start=(n == 0), stop=(n == NC_CHUNKS * K - 1))
                    n += 1
            squash_from(ps[:])

        def s_round_weighted(write_out):
            """s_raw via e-weighted matmuls with diag extract; squash with Z."""
            psA_t = psum_s.tile([128, 8 * B], F32, tag="ps_sA", name="psA_t")
            psB_t = psum_s.tile([32, 2 * B], F32, tag="ps_sB", name="psB_t")
            psA = psA_t[:]                  # [(j'8,d),(j0..7,b)]
            psB = psB_t[:]                  # [(j'2,d),(j8..9,b)]
            n = 0
            for c in range(NC_CHUNKS):
                # exk[(k,j,b)] = e[c][(j,b)] * xs[c][(k,b)]  (one fused op)
                exk = exk_p.tile([128, K * J * B], F16, tag="exk")
                e_src = bass.AP(tensor=e_t[c].tensor, offset=e_t[c].offset,
                                ap=[e_t[c].ap[0], [0, K], [B, J], [1, B]])
                x_base = xs_t[:, c * K * B:(c + 1) * K * B]
                x_src = bass.AP(tensor=x_base.tensor, offset=x_base.offset,
                                ap=[x_base.ap[0], [B, K], [0, J], [1, B]])
                eng = nc.gpsimd if c in EXK_POOL_CHUNKS else nc.vector
                eng.tensor_tensor(
                    out=exk[:].rearrange("p (k j b) -> p k j b", k=K, j=J),
                    in0=e_src, in1=x_src, op=ALU.mult)
                for k in range(K):
                    st_ = (n == 0)
                    sp = (n == NC_CHUNKS * K - 1)
                    wck = ws_ck(c, k)
                    o = k * J * B
                    nc.tensor.matmul(psA, wck[:, 0:128], exk[:, o:o + 8 * B],
                                     start=st_, stop=sp)
                    nc.tensor.matmul(psB, wck[:, 128:160],
                                     exk[:, o + 8 * B:o + J * B],
                                     start=st_, stop=sp)
                    n += 1
            # diag extract -> s-panels [(j,d), b] -> PE transpose -> sraw
            psA_s = small.tile([128, 8 * B], F32, tag="psA_s")
            nc.vector.tensor_copy(psA_s[:], psA)
            psB_s = small.tile([32, 2 * B], F32, tag="psB_s")
            nc.vector.tensor_copy(psB_s[:], psB)
            spanA = small.tile([128, B], F32, tag="spanA")
            spanB = small.tile([32, B], F32, tag="spanB")
            for jp in range(8):
                eng = nc.sync if jp % 2 == 0 else nc.scalar
                eng.dma_start(
                    spanA[16 * jp:16 * (jp + 1), :],
                    psA_s[16 * jp:16 * (jp + 1), jp * B:(jp + 1) * B])
            for jp in range(2):
                eng = nc.sync if jp % 2 == 0 else nc.scalar
                eng.dma_start(
                    spanB[16 * jp:16 * (jp + 1), :],
                    psB_s[16 * jp:16 * (jp + 1), jp * B:(jp + 1) * B])
            stA = psum_t.tile([B, 128], F32, tag="at", name="stA")
            nc.tensor.transpose(stA[:], spanA[:], identF[:, :])
            stB = psum_t.tile([B, 32], F32, tag="at", name="stB")
            nc.tensor.transpose(stB[:], spanB[:], ident32[:, :])
            sraw = small.tile([B, JD], F32, tag="sraw")
            nc.vector.tensor_copy(sraw[:, 0:128], stA[:])
            nc.vector.tensor_copy(sraw[:, 128:160], stB[:])
            squash_from(sraw[:])
            if write_out:
                nc.sync.dma_start(
                    out_d[:, :, :].rearrange("b j d -> b (j d)"), vpan[:])

        # ================= program =================
        nc.vector.memset(z_jb, float(I))   # Z = I for the uniform round
        s_round_uniform()          # -> vpan = v1
        v_to_vbd()
        stage1_and_a(a1, None)     # a1
        exp_and_z(a1)              # e = exp(a1 - max), Z
        s_round_weighted(False)    # -> vpan = v2
        v_to_vbd()
        stage1_and_a(a2, a1)       # a2 = a1 + delta
        exp_and_z(a2)
        s_round_weighted(True)     # -> v3 -> out

    nc.finalize()
    return nc


def _prep_inputs(x_full, w_full):
    """Host-side layout prep (numpy, layout only). Returns per-core in_maps."""
    W = w_full  # [I, J, D, K]
    # wp[g]: [(j_loc,d), (i,k)] fp16
    wp = []
    for (j0, nj) in JG:
        wpg = W[:, j0:j0 + nj, :, :].transpose(1, 2, 3, 0).reshape(nj * D, IK)
        wp.append(np.ascontiguousarray(wpg, dtype=np.float16))
    # ws: [(i)128, c, k, (j,d)] fp16
    ws = W.reshape(NC_CHUNKS, 128, J, D, K).transpose(1, 0, 4, 2, 3)
    ws = np.ascontiguousarray(ws.reshape(128, NC_CHUNKS * K * JD), dtype=np.float16)

    in_maps = []
    for c in range(N_CORES):
        xb = x_full[c * B:(c + 1) * B]           # [32, I, K]
        xs = xb.reshape(B, NC_CHUNKS, 128, K).transpose(2, 1, 3, 0)  # [i,c,k,b]
        xs = np.ascontiguousarray(xs.reshape(128, NC_CHUNKS * K * B),
                                  dtype=np.float16)
        xki = xb.transpose(0, 2, 1).reshape(B, IK)      # [b, (k,i)]
        xrep = np.tile(xki, (4, 1)).astype(np.float16)
        m = {"ws": ws, "xs": xs, "xrep": np.ascontiguousarray(xrep)}
        for g in range(3):
            m[f"wp{g}"] = wp[g]
        in_maps.append(m)
    return in_maps


def kernel(x, W):
    """x: [256, 1152, 8] f32, W: [1152, 10, 16, 8] f32 -> [256, 10, 16] f32."""
    x = np.asarray(x, dtype=np.float32)
    W = np.asarray(W, dtype=np.float32)
    if "nc" not in _CACHE:
        _CACHE["nc"] = _build_nc()
    nc = _CACHE["nc"]
    in_maps = _prep_inputs(x, W)
    res = run_bass_kernel_spmd(nc, in_maps, core_ids=list(range(N_CORES)))
    outs = [r["out"] for r in res.results]
    return np.concatenate(outs, axis=0)
